# revision 28
# baseline (speedup 1.0000x reference)
"""Deformable alignment fusion kernel for TRN2, 8-core data-parallel.

Math (per batch b):
  cat    = concat([low, high], ch)                       # (256, H, W)
  offset = conv3x3(cat, w_off) + b_off                   # (18, H, W)  (dy,dx)*9 taps
  aligned= deform_conv(low, offset, w_def) + b_def       # (128, H, W)
  gate   = sigmoid(w_mod @ cat + b_mod)                  # (128, H, W)
  out    = aligned * gate + high

Sharding: core i handles batch b = i//2, rows [64*(i%2), 64*(i%2)+64).

Device algorithm per core (channel-major, fp16 matmul operands):
 - per-quarter software pipeline: offset conv -> index math -> gathers ->
   deform matmuls, so the Q7 gathers of quarter q overlap the PE work of
   adjacent quarters.
 - offset conv: direct 3x3 conv as 18 accumulating matmuls per 512-px chunk.
 - bilinear sampling in "monomial" form: S = P0 + wx*P1 + wy*P2 + wx*wy*P3
   where P0..P3 are the value / x-diff / y-diff / xy-diff planes of the
   (guard-padded) low image, all gathered at the single flat index
   i0 = floor(py)*136 + floor(px) from a host-prepared pixel-major
   4-plane table via dma_gather(transpose=True)  -> channel-major tiles.
 - i0 itself is built by 4 accumulating matmuls with tiny diagonal lhsT
   tiles: psum = HP*py + px - (HP*fracy + fracx), evacuated to int16 by
   the scalar engine (round-to-nearest cast).
 - per-pixel weights wx, wy are broadcast to 128 partitions on the TENSOR
   engine (selector-row matmul into PSUM) and evacuated to fp16 SBUF by
   the scalar engine; wx*wy is computed on DVE from the fp16 tiles.
   (The old gpsimd partition_broadcast was the kernel bottleneck: Q7 work
   serialized with gather descriptor generation.)
 - the deform conv contraction folds the monomial sum into PSUM:
   psum += w_def_k.T @ P0 ; += w_def_k.T @ (wx*P1) ; ...  (4 matmuls/tap)
 - gate: two 1x1 matmuls + Sigmoid on the scalar engine.
"""

import numpy as np

DEBUG_DUMPS = False

import concourse.bass as bass
import concourse.tile as tile
from concourse import bacc, mybir
from concourse.bass import ts

dt = mybir.dt
F16 = dt.float16
F32 = dt.float32
I16 = dt.int16
Alu = mybir.AluOpType
Act = mybir.ActivationFunctionType

B, C, H, W = 4, 128, 128, 128
GP = 4                 # guard pad for sampling
HP = H + 2 * GP        # 136
NP = HP * HP           # 18496 padded pixels
HR = 64                # rows per core
NPIX = HR * W          # 8192 pixels per core
NQ = 4                 # quarters per core
QP = NPIX // NQ        # 2048 pixels per quarter
QR = QP // W           # 16 rows per quarter

# offset channel regrouping: rows 0..8 = dy(tap), rows 9..17 = dx(tap)
PERM = [2 * k for k in range(9)] + [2 * k + 1 for k in range(9)]

MAGIC_A = 8388608.0 - 0.5
MAGIC_B = 8388608.0


def _ap(t, offset, dims):
    """Raw AP on the same tensor as AP `t`, with explicit [step, count] dims."""
    return bass.AP(tensor=t.tensor, offset=t.offset + offset, ap=list(dims))


def build_program():
    nc = bacc.Bacc("TRN2", debug=False)

    io = {}

    def din(name, shape, d):
        io[name] = nc.dram_tensor(name, shape, d, kind="ExternalInput").ap()
        return io[name]

    din("src4", [NP, 512], F16)           # [pix, (4 planes x 128 ch)]
    din("lowp", [128, 66 * 130], F16)     # rows h0-1..h1+1, W-padded by 1
    din("highp", [128, 66 * 130], F16)
    din("highc", [128, NPIX], F32)        # center high rows, f32
    din("w_off_t", [2, 3, 3, 128, 18], F16)
    din("w_def_t", [9, 128, 128], F16)
    din("w_mod_t", [2, 128, 128], F16)
    din("b_off_g", [18, 1], F32)
    din("b_def_c", [128, 1], F32)
    din("b_mod_c", [128, 1], F32)
    din("base_w", [18, NPIX], F16)        # sampling-position bases, wrapped order
    din("sel9", [9, 9, 128], F16)         # sel9[r,k,:] = (r==k)
    din("i9u", [9, 2, 9], F16)            # diag(HP) | diag(1)  (i0 = HP*fy+fx)
    io["idx_scr"] = nc.dram_tensor("idx_scr", [9, NPIX], I16, kind="Internal").ap()
    if DEBUG_DUMPS:
        io["dbg_idx"] = nc.dram_tensor("dbg_idx", [9, NPIX], I16, kind="ExternalOutput").ap()
        io["dbg_fracx"] = nc.dram_tensor("dbg_fracx", [9, NPIX], F16, kind="ExternalOutput").ap()
        io["dbg_fracy"] = nc.dram_tensor("dbg_fracy", [9, NPIX], F16, kind="ExternalOutput").ap()
        io["dbg_wbc"] = nc.dram_tensor("dbg_wbc", [128, 2, 2048], F16, kind="ExternalOutput").ap()
        io["dbg_gate"] = nc.dram_tensor("dbg_gate", [128, NPIX], F16, kind="ExternalOutput").ap()
    out_d = nc.dram_tensor("out", [128, NPIX], F32, kind="ExternalOutput").ap()

    with tile.TileContext(nc) as tc:
        trace_kernel(tc, io, out_d)

    nc.compile()
    return nc


def trace_kernel(tc, io, out_d):
    nc = tc.nc
    from contextlib import ExitStack

    ctx = ExitStack()
    consts = ctx.enter_context(tc.tile_pool(name="consts", bufs=1))
    # psum: shared ring (stage A + weight-broadcast) 2x[128,2,512] = 4 banks,
    # deform accumulator [128,2048] = 4 banks.
    ring = ctx.enter_context(tc.tile_pool(name="ps_ring", bufs=2, space="PSUM"))
    dpool = ctx.enter_context(tc.tile_pool(name="ps_deform", bufs=1, space="PSUM"))
    # per-quarter sbuf tiles
    qpool1 = ctx.enter_context(tc.tile_pool(name="qtmp1", bufs=1))
    qpool2 = ctx.enter_context(tc.tile_pool(name="qtmp2", bufs=2))
    gpool = ctx.enter_context(tc.tile_pool(name="gather", bufs=4))
    tpool = ctx.enter_context(tc.tile_pool(name="tplanes", bufs=2))
    wpool = ctx.enter_context(tc.tile_pool(name="wbc", bufs=2))
    spool = ctx.enter_context(tc.tile_pool(name="small", bufs=2))

    # ---------------- constants to SBUF ----------------
    w_off_sb = consts.tile([128, 2, 3, 3, 18], F16)
    nc.sync.dma_start(
        w_off_sb[:], io["w_off_t"].rearrange("cb ky kx c o -> c cb ky kx o")
    )
    w_def_sb = consts.tile([128, 9, 128], F16)
    nc.sync.dma_start(w_def_sb[:], io["w_def_t"].rearrange("k c o -> c k o"))
    w_mod_sb = consts.tile([128, 2, 128], F16)
    nc.sync.dma_start(w_mod_sb[:], io["w_mod_t"].rearrange("cb c o -> c cb o"))
    b_off_sb = consts.tile([18, 1], F32)
    nc.sync.dma_start(b_off_sb[:], io["b_off_g"])
    b_def_sb = consts.tile([128, 1], F32)
    nc.sync.dma_start(b_def_sb[:], io["b_def_c"])
    b_mod_sb = consts.tile([128, 1], F32)
    nc.sync.dma_start(b_mod_sb[:], io["b_mod_c"])
    sel9_sb = consts.tile([9, 9, 128], F16)
    nc.sync.dma_start(sel9_sb[:], io["sel9"])
    i9u_sb = consts.tile([9, 2, 9], F16)
    nc.sync.dma_start(i9u_sb[:], io["i9u"])
    zero9 = consts.tile([9, 512], F16)
    nc.vector.memset(zero9[:], 0.0)
    # warm the PE HAM clock before the first offset conv (~5us busy)
    for i in range(12):
        psw = ring.tile([128, 2, 512], F32, tag="ring")
        nc.tensor.matmul(
            psw[:, 0, :], lhsT=sel9_sb[0:9, 0, :], rhs=zero9[:],
            start=True, stop=True,
        )

    # per-quarter state (produced by stage AB(q), consumed by D(q)/E(q))
    state = {}

    def stage_AB(q):
        r0g = q * QR  # first image row of this quarter (0..48)
        # padded-image tiles for this quarter: rows r0g..r0g+18 of the
        # 66-row padded strip, all 130 cols
        lowq = qpool2.tile([128, 18, 130], F16, tag="lowq")
        nc.sync.dma_start(
            lowq[:],
            _ap(io["lowp"], r0g * 130, [[66 * 130, 128], [130, 18], [1, 130]]),
        )
        highq = qpool2.tile([128, 18, 130], F16, tag="highq")
        nc.sync.dma_start(
            highq[:],
            _ap(io["highp"], r0g * 130, [[66 * 130, 128], [130, 18], [1, 130]]),
        )
        base_q = qpool2.tile([18, QP], F16, tag="baseq")
        nc.sync.dma_start(base_q[:], io["base_w"][:, ts(q, QP)])

        # ---- offset conv: pos = psum + b_off + base (wrapped order) ----
        pos_q = qpool1.tile([18, QP], F32, tag="posq")
        for cc in range(4):
            ps = ring.tile([128, 2, 512], F32, tag="ring")
            r0 = cc * 4
            n_mm = 0
            for cb in range(2):
                pad = lowq if cb == 0 else highq
                for ky in range(3):
                    for kx in range(3):
                        nc.tensor.matmul(
                            ps[0:18, 0, :],
                            lhsT=w_off_sb[:, cb, ky, kx, :],
                            rhs=pad[:, r0 + ky : r0 + ky + 4, kx : kx + 128],
                            start=(n_mm == 0),
                            stop=(n_mm == 17),
                        )
                        n_mm += 1
            # evacuate with wrapped reorder: dest (p16, s) <- psum col s*16+p16
            ps3 = ps[0:18, 0, :].rearrange("r (s p) -> r p s", p=16)  # [18,16,32]
            dest = pos_q[:].rearrange("r (p s) -> r p s", p=16)[
                :, :, cc * 32 : cc * 32 + 32
            ]
            base3 = base_q[:].rearrange("r (p s) -> r p s", p=16)[
                :, :, cc * 32 : cc * 32 + 32
            ]
            nc.vector.scalar_tensor_tensor(
                out=dest, in0=ps3, scalar=b_off_sb[:], in1=base3,
                op0=Alu.add, op1=Alu.add,
            )

        # ---- narrow index/weight math ----
        # move px rows 9..17 -> partitions 0..8 (HWDGE sbuf->sbuf)
        posx_q = qpool1.tile([9, QP], F32, tag="posxq")
        nc.sync.dma_start(posx_q[:], pos_q[9:18, :])
        # floor via magic round: fy = round(pos - 0.5), integer-valued
        # <= 134 so exact in f16; frac = pos - fy.
        fy16 = qpool1.tile([9, QP], F16, tag="fy16q")
        nc.vector.tensor_scalar(
            fy16[:], pos_q[0:9, :], MAGIC_A, MAGIC_B, Alu.add, Alu.subtract
        )
        fracy = qpool2.tile([9, QP], F16, tag="fracy")
        nc.vector.tensor_tensor(fracy[:], pos_q[0:9, :], fy16[:], Alu.subtract)
        fx16 = qpool1.tile([9, QP], F16, tag="fx16q")
        nc.vector.tensor_scalar(
            fx16[:], posx_q[:], MAGIC_A, MAGIC_B, Alu.add, Alu.subtract
        )
        fracx = qpool2.tile([9, QP], F16, tag="fracx")
        nc.vector.tensor_tensor(fracx[:], posx_q[:], fx16[:], Alu.subtract)

        # i0 = HP*fy + fx: f16 x f16 integer products accumulate exactly in
        # the f32 psum, so the int16 evacuation cast is exact.
        idx16 = qpool2.tile([9, QP], I16, tag="idx16")
        for cc in range(4):
            sl = ts(cc, 512)
            psu = ring.tile([128, 2, 512], F32, tag="ring")
            nc.tensor.matmul(
                psu[0:9, 0, :], lhsT=i9u_sb[:, 0, :], rhs=fy16[:, sl],
                start=True, stop=False, skip_group_check=True,
            )
            nc.tensor.matmul(
                psu[0:9, 0, :], lhsT=i9u_sb[:, 1, :], rhs=fx16[:, sl],
                start=False, stop=True, skip_group_check=True,
            )
            # psum holds int +- ~0.07 fp noise; evacuate via the same DVE
            # scalar_tensor_tensor form the old kernel used — its int16
            # output cast rounds to nearest on both sim and HW (plain
            # tensor_scalar truncates in sim; ACT rounds only on HW).
            nc.vector.scalar_tensor_tensor(
                out=idx16[:, sl], in0=psu[0:9, 0, :], scalar=0.0,
                in1=zero9[:], op0=Alu.add, op1=Alu.add,
            )

        # wrapped idx rows -> DRAM -> replicated to all 8 Q7 core groups
        nc.sync.dma_start(
            _ap(io["idx_scr"], q * QP, [[NPIX, 9], [1, QP]]), idx16[:]
        )
        idxr = qpool2.tile([128, 9, 128], I16, tag="idxr")
        for k in range(9):
            rep_ap = _ap(
                io["idx_scr"], k * NPIX + q * QP,
                [[0, 8], [128, 16], [1, 128]],
            )
            nc.sync.dma_start(idxr[:, k, :], rep_ap)

        # ---- gate for this quarter (natural pixel order) ----
        gate_q = qpool2.tile([128, QP], F16, tag="gateq")
        for cc in range(4):
            psg = ring.tile([128, 2, 512], F32, tag="ring")
            for cb in range(2):
                pad = lowq if cb == 0 else highq
                nc.tensor.matmul(
                    psg[:, 0, :],
                    lhsT=w_mod_sb[:, cb, :],
                    rhs=pad[:, 1 + cc * 4 : 1 + cc * 4 + 4, 1:129],
                    start=(cb == 0),
                    stop=(cb == 1),
                )
            nc.scalar.activation(
                out=gate_q[:, ts(cc, 512)], in_=psg[:, 0, :],
                func=Act.Sigmoid, bias=b_mod_sb[:], scale=1.0,
            )

        if DEBUG_DUMPS:
            nc.sync.dma_start(
                _ap(io["dbg_idx"], q * QP, [[NPIX, 9], [1, QP]]), idx16[:]
            )
            nc.sync.dma_start(
                _ap(io["dbg_fracx"], q * QP, [[NPIX, 9], [1, QP]]), fracx[:]
            )
            nc.sync.dma_start(
                _ap(io["dbg_fracy"], q * QP, [[NPIX, 9], [1, QP]]), fracy[:]
            )
            nc.sync.dma_start(io["dbg_gate"][:, ts(q, QP)], gate_q[:])

        state[q] = dict(
            gate_q=gate_q, fracx=fracx, fracy=fracy, idxr=idxr
        )

    def unwrap_rhs(frac, cc):
        """Natural-order 512-col view of a wrapped [9, QP] frac tile.

        natural pixel j = cc*512 + 16a + b  lives at wrapped col b*128+32cc+a,
        so stream a (stride 1) outer and b (stride 128) inner.
        """
        return _ap(frac, 32 * cc, [list(frac.ap[0]), [1, 32], [128, 16]])

    def stage_D(q):
        st = state[q]
        dps = dpool.tile([128, QP], F32)  # 4 PSUM banks

        def produce(k):
            """gather + weight broadcast for tap k (no G-dependent PE work)"""
            G = gpool.tile([128, 4, QP], F16, tag="g")
            nc.gpsimd.dma_gather(
                out_ap=G[:],
                in_ap=io["src4"],
                idxs_ap=st["idxr"][:, k, :],
                num_idxs=QP,
                num_idxs_reg=QP,
                elem_size=512,
                transpose=True,
                single_packet=False,
            )
            wbc = wpool.tile([128, 2, 4, 512], F16, tag="w")
            for cc in range(4):
                psb = ring.tile([128, 2, 512], F32, tag="ring")
                nc.tensor.matmul(
                    psb[:, 0, :], lhsT=sel9_sb[:, k, :],
                    rhs=unwrap_rhs(st["fracx"], cc),
                    start=True, stop=True,
                )
                nc.tensor.matmul(
                    psb[:, 1, :], lhsT=sel9_sb[:, k, :],
                    rhs=unwrap_rhs(st["fracy"], cc),
                    start=True, stop=True,
                )
                nc.scalar.activation(
                    out=wbc[:, :, cc, :], in_=psb[:], func=Act.Copy,
                )
            return G, wbc

        def consume(k, G, wbc):
            """T-planes + deform matmuls for tap k (needs G)"""
            wx = wbc[:, 0, :, :].rearrange("c q s -> c (q s)")
            wy = wbc[:, 1, :, :].rearrange("c q s -> c (q s)")
            T = tpool.tile([128, 3, QP], F16, tag="t")
            nc.vector.tensor_tensor(T[:, 0, :], G[:, 1, :], wx, Alu.mult)
            nc.vector.tensor_tensor(T[:, 1, :], G[:, 2, :], wy, Alu.mult)
            nc.vector.tensor_tensor(T[:, 2, :], wx, wy, Alu.mult)
            nc.vector.tensor_tensor(T[:, 2, :], G[:, 3, :], T[:, 2, :], Alu.mult)
            for cc in range(4):
                sl = ts(cc, 512)
                for plane, rhs in enumerate(
                    (G[:, 0, sl], T[:, 0, sl], T[:, 1, sl], T[:, 2, sl])
                ):
                    nc.tensor.matmul(
                        dps[:, sl],
                        lhsT=w_def_sb[:, k, :],
                        rhs=rhs,
                        start=(k == 0 and plane == 0),
                        stop=(k == 8 and plane == 3),
                    )

        # one-tap software pipeline: tap k+1's gather/broadcasts are issued
        # before tap k's G-dependent matmuls, so the PE never idles behind
        # a gather at the head of its in-order queue.
        prev = produce(0)
        for k in range(1, 9):
            nxt = produce(k)
            consume(k - 1, *prev)
            prev = nxt
        consume(8, *prev)
        return dps

    def stage_E(q, dps):
        st = state[q]
        for cc in range(4):
            gsl = ts(q * 4 + cc, 512)
            t1 = spool.tile([128, 512], F32, tag="as1")
            nc.vector.scalar_tensor_tensor(
                out=t1[:], in0=dps[:, ts(cc, 512)], scalar=b_def_sb[:],
                in1=st["gate_q"][:, ts(cc, 512)], op0=Alu.add, op1=Alu.mult,
            )
            hc = spool.tile([128, 512], F32, tag="hc")
            nc.sync.dma_start(hc[:], io["highc"][:, gsl])
            nc.vector.tensor_tensor(t1[:], t1[:], hc[:], Alu.add)
            nc.sync.dma_start(out_d[:, gsl], t1[:])
        del state[q]

    # software pipeline: AB one quarter ahead of D/E
    stage_AB(0)
    for q in range(NQ):
        if q + 1 < NQ:
            stage_AB(q + 1)
        dps = stage_D(q)
        stage_E(q, dps)

    ctx.close()


# ======================= host side =======================

def _prep_shared(w_off, b_off, w_def, b_def, w_mod, b_mod):
    w_off_g = w_off[PERM]                      # [18, 256, 3, 3]
    w_off_t = np.ascontiguousarray(
        w_off_g.reshape(18, 2, 128, 3, 3).transpose(1, 3, 4, 2, 0)
    ).astype(np.float16)                       # [2,3,3,128,18]
    b_off_g = b_off[PERM].reshape(18, 1).astype(np.float32)
    w_def_t = np.ascontiguousarray(
        w_def.reshape(128, 128, 9).transpose(2, 1, 0)
    ).astype(np.float16)                       # [9, c, o]
    w_mod_t = np.ascontiguousarray(
        w_mod.reshape(128, 2, 128).transpose(1, 2, 0)
    ).astype(np.float16)                       # [2, c, o]
    sel9 = np.zeros((9, 9, 128), np.float16)
    for k in range(9):
        sel9[k, k, :] = 1.0
    i9u = np.zeros((9, 2, 9), np.float16)
    for r in range(9):
        i9u[r, 0, r] = float(HP)
        i9u[r, 1, r] = 1.0
    return dict(
        w_off_t=w_off_t,
        b_off_g=b_off_g,
        w_def_t=w_def_t,
        b_def_c=b_def.reshape(128, 1).astype(np.float32),
        w_mod_t=w_mod_t,
        b_mod_c=b_mod.reshape(128, 1).astype(np.float32),
        sel9=sel9,
        i9u=i9u,
    )


def _prep_src4(low_b):
    """4-plane pixel-major monomial table of the guard-padded low image."""
    xp = np.zeros((C, HP, HP), np.float32)
    xp[:, GP : GP + H, GP : GP + W] = low_b
    f = xp.reshape(C, NP)
    p0 = f
    p1 = np.zeros_like(f)
    p1[:, :-1] = f[:, 1:] - f[:, :-1]
    p2 = np.zeros_like(f)
    p2[:, :-HP] = f[:, HP:] - f[:, :-HP]
    p3 = np.zeros_like(f)
    p3[:, : -HP - 1] = f[:, HP + 1 :] - f[:, HP:-1] - f[:, 1 : -HP] + f[:, : -HP - 1]
    planes = np.stack([p0, p1, p2, p3], 0)      # [4, C, NP]
    return np.ascontiguousarray(planes.transpose(2, 0, 1)).astype(
        np.float16
    ).reshape(NP, 512)


def _prep_base(h0):
    """Sampling-position bases in wrapped order, rows grouped [9 x py, 9 x px]."""
    base = np.empty((18, NPIX), np.float32)
    jj = np.arange(NPIX)
    # wrapped order: within quarter q, column t = p16*128 + s holds the pixel
    # j = s*16 + p16 (local to the quarter)
    q = jj // QP
    tloc = jj % QP
    p16 = tloc // 128
    s = tloc % 128
    jloc = s * 16 + p16
    h = h0 + (q * QP + jloc) // W
    w = (q * QP + jloc) % W
    for k in range(9):
        ky, kx = k // 3, k % 3
        base[k] = h + (ky - 1) + GP
        base[9 + k] = w + (kx - 1) + GP
    return base.astype(np.float16)


def _prep_core(low_b, high_b, h0):
    lp = np.pad(low_b, ((0, 0), (1, 1), (1, 1)))
    hp = np.pad(high_b, ((0, 0), (1, 1), (1, 1)))
    lowp = np.ascontiguousarray(lp[:, h0 : h0 + 66, :]).reshape(128, -1).astype(
        np.float16
    )
    highp = np.ascontiguousarray(hp[:, h0 : h0 + 66, :]).reshape(128, -1).astype(
        np.float16
    )
    highc = np.ascontiguousarray(high_b[:, h0 : h0 + HR, :]).reshape(128, -1).astype(
        np.float32
    )
    return lowp, highp, highc


_PROGRAM_CACHE = {}
_LAST_IN_MAPS = None


def make_in_maps(low_res, high_res, w_off, b_off, w_def, b_def, w_mod, b_mod):
    shared = _prep_shared(
        np.asarray(w_off, np.float32), np.asarray(b_off, np.float32),
        np.asarray(w_def, np.float32), np.asarray(b_def, np.float32),
        np.asarray(w_mod, np.float32), np.asarray(b_mod, np.float32),
    )
    low_res = np.asarray(low_res, np.float32)
    high_res = np.asarray(high_res, np.float32)
    src4_by_batch = [_prep_src4(low_res[b]) for b in range(B)]
    in_maps = []
    for core in range(8):
        b, half = core // 2, core % 2
        h0 = half * HR
        lowp, highp, highc = _prep_core(low_res[b], high_res[b], h0)
        m = dict(shared)
        m["src4"] = src4_by_batch[b]
        m["lowp"] = lowp
        m["highp"] = highp
        m["highc"] = highc
        m["base_w"] = _prep_base(h0)
        in_maps.append(m)
    return in_maps


def kernel(low_res, high_res, w_off, b_off, w_def, b_def, w_mod, b_mod):
    global _LAST_IN_MAPS
    if "nc" not in _PROGRAM_CACHE:
        _PROGRAM_CACHE["nc"] = build_program()
    nc = _PROGRAM_CACHE["nc"]

    in_maps = make_in_maps(
        low_res, high_res, w_off, b_off, w_def, b_def, w_mod, b_mod
    )
    _LAST_IN_MAPS = in_maps

    from concourse import bass_utils

    res = bass_utils.run_bass_kernel_spmd(nc, in_maps, core_ids=list(range(8)))
    out = np.empty((B, C, H, W), np.float32)
    for core in range(8):
        b, half = core // 2, core % 2
        out[b, :, half * HR : half * HR + HR, :] = (
            res.results[core]["out"].reshape(C, HR, W)
        )
    return out
